# revision 23
# baseline (speedup 1.0000x reference)
"""ArcFace loss with adaptive margins and subcenters, distributed over 8 TRN2 cores.

Problem: features [512, 512] f32, weight [300000, 512] f32 (100000 classes x 3
subcenters), margins [100000] f32, labels [512] int. Output [512, 100000] f32:
S * max_k cos(f, w_{c,k}) everywhere, with the ArcFace margin phi at each
sample's label column.

Strategy (classifier/model parallel, per the class-sharding hint):
  - Host: L2-normalize features and weights, fold the scale S into the weight,
    cast to fp16, and pack each core's 12500-class shard into a DMA-friendly
    layout. Also compute (exactly, in f32) the per-sample label-column value
    phi, since that needs only 512 dot products.
  - Device (x8, no collectives needed): stream the packed weight shard from
    HBM, 3 GEMMs per class-chunk (one per subcenter) accumulating over the
    D=512 contraction in PSUM, elementwise max over the 3 subcenters on the
    vector engine, write the S-scaled cosine shard back to HBM.
  - Host: concatenate the 8 [512, 12500] shards and overwrite the 512 label
    entries with S*phi.

Per-core roofline: 9.8 GMAC -> ~253 us on the PE at fp16; 64 MB of HBM
traffic -> ~180 us. PE-bound at ~260 us if DMA overlaps.
"""

import numpy as np

B = 512            # batch
D = 512            # in_features
C = 100000         # n_classes
K = 3              # subcenters
S = 30.0           # ArcFace scale
NCORES = 8
CPC = C // NCORES  # classes per core = 12500
NCHUNK = 500       # output columns per PSUM tile
CHUNKS = CPC // NCHUNK   # 25
NB = B // 128      # 4 row blocks of the batch
DBLK = D // 128    # 4 contraction blocks

_CACHE = {}
LAST_RESULT = None  # BassKernelResults of the most recent run (for profiling)


def _install_profile_hook():
    """Make `antenv.axon_hooks` importable (concourse imports it when tracing
    is requested via BASS_TRACE) and register the NTFF hook if available."""
    import sys
    import types
    try:
        import antenv
    except ImportError:
        return
    if getattr(antenv, "axon_hooks", None) is not None:
        return
    mod = types.ModuleType("antenv.axon_hooks")
    _hook = [None]
    mod.set_axon_ntff_profile_hook = lambda h: _hook.__setitem__(0, h)
    mod.get_axon_ntff_profile_hook = lambda: _hook[0]
    sys.modules["antenv.axon_hooks"] = mod
    antenv.axon_hooks = mod
    try:
        from trn_agent_boot.trn_boot import _ntff_profile_via_ctypes
        hook = _ntff_profile_via_ctypes("/opt/axon/libaxon_pjrt.so")
        if hook is not None:
            mod.set_axon_ntff_profile_hook(hook)
    except Exception:
        pass


def _build_nc():
    if "nc" in _CACHE:
        return _CACHE["nc"]
    import concourse.bacc as bacc
    import concourse.tile as tile
    from concourse import mybir

    BF = mybir.dt.float16
    F32 = mybir.dt.float32

    nc = bacc.Bacc("TRN2", target_bir_lowering=False, debug=False, num_devices=NCORES)
    # Packed weight shard: wt[q][p][(k*DBLK+d)*NCHUNK + j] = S * wn[3*(c0+q*500+j)+k, d*128+p]
    wt = nc.dram_tensor("wt", [CHUNKS, 128, K * DBLK * NCHUNK], BF, kind="ExternalInput")
    # Normalized features, transposed: fnT[d][p][b] = fn[b, d*128+p]
    fnT = nc.dram_tensor("fnT", [DBLK, 128, B], BF, kind="ExternalInput")
    out = nc.dram_tensor("out", [B, CPC], F32, kind="ExternalOutput")

    with tile.TileContext(nc, trace_sim=False) as tc:
        with tc.tile_pool(name="fp", bufs=1) as fpool, \
             tc.tile_pool(name="wp", bufs=4) as wpool, \
             tc.tile_pool(name="op", bufs=3) as opool, \
             tc.tile_pool(name="tp", bufs=4) as tpool, \
             tc.tile_pool(name="pp", bufs=2, space="PSUM") as ppool:
            f_sb = fpool.tile([128, DBLK * B], BF)
            for q in range(CHUNKS):
                w_sb = wpool.tile([128, K * DBLK * NCHUNK], BF)
                if q == 0:
                    # Split the first chunk's load into per-(k,d) slices,
                    # issued in the order the matmuls consume them (d-outer,
                    # k-inner, interleaved with the feature tiles) across
                    # both HWDGE rings, so the first matmul only waits for
                    # its own 128 KB slice.
                    wt3 = wt[q].rearrange("p (k d j) -> p k d j", k=K, d=DBLK)
                    for d in range(DBLK):
                        nc.scalar.dma_start(f_sb[:, d * B:(d + 1) * B], fnT[d])
                        for k in range(K):
                            j = k * DBLK + d
                            eng = nc.sync if k != 1 else nc.scalar
                            eng.dma_start(
                                w_sb[:, j * NCHUNK:(j + 1) * NCHUNK], wt3[:, k, d]
                            )
                else:
                    nc.sync.dma_start(w_sb[:], wt[q])
                for b in range(NB):
                    ps = [
                        ppool.tile([128, NCHUNK], F32, tag=f"ps{k}", name=f"ps{k}")
                        for k in range(K)
                    ]
                    # d-outer / k-inner: the stationary operand (features)
                    # is reused across the 3 subcenter matmuls.
                    for d in range(DBLK):
                        lh = f_sb[:, d * B + b * 128: d * B + (b + 1) * 128]
                        for k in range(K):
                            rh = w_sb[:, (k * DBLK + d) * NCHUNK:(k * DBLK + d + 1) * NCHUNK]
                            nc.tensor.matmul(
                                ps[k][:], lh, rh,
                                start=(d == 0), stop=(d == DBLK - 1),
                                skip_group_check=True,
                            )
                    # DVE can't read two PSUM banks in one op; stage k=0
                    # through SBUF on the (otherwise idle) scalar engine.
                    t0 = tpool.tile([128, NCHUNK], F32, tag="t0", name="t0")
                    nc.scalar.copy(t0[:], ps[0][:])
                    t01 = tpool.tile([128, NCHUNK], F32, tag="t01", name="t01")
                    nc.vector.tensor_max(t01[:], t0[:], ps[1][:])
                    ob = opool.tile([128, NCHUNK], F32, tag=f"ob{b}", name=f"ob{b}")
                    nc.vector.tensor_max(ob[:], t01[:], ps[2][:])
                    # Output stores go on the scalar engine's HWDGE ring so
                    # they don't queue ahead of weight prefetches on sync's.
                    nc.scalar.dma_start(
                        out[b * 128:(b + 1) * 128, q * NCHUNK:(q + 1) * NCHUNK],
                        ob[:],
                    )
    nc.compile()
    _CACHE["nc"] = nc
    return nc


def _to_f16(x):
    # fp16 storage/compute: same TensorE rate and DMA bytes as bf16, but a
    # 10-bit mantissa -> ~4x less quantization error. All values here are
    # bounded by S=30, far inside fp16 range.
    return np.asarray(x, np.float32).astype(np.float16)


def kernel(features, weight, margins, labels):
    global LAST_RESULT
    from concourse.bass_utils import run_bass_kernel_spmd

    feats = np.asarray(features, np.float32)
    w = np.asarray(weight, np.float32)
    marg = np.asarray(margins, np.float32)
    lab = np.asarray(labels).astype(np.int64)

    nc = _build_nc()

    # --- host prep: normalize, fold S, pack per core ---
    fn = feats / np.linalg.norm(feats, axis=1, keepdims=True)
    fnT_f16 = np.ascontiguousarray(_to_f16(fn.T).reshape(DBLK, 128, B))

    R = CPC * K  # weight rows per core
    in_maps = []
    for m in range(NCORES):
        rows = w[m * R:(m + 1) * R]
        nrm = np.sqrt(np.einsum("ij,ij->i", rows, rows, dtype=np.float32))
        arr = _to_f16(rows * (S / nrm)[:, None])
        # [3c+k, d] -> [q, p, k, d, j]
        pack = np.ascontiguousarray(
            arr.reshape(CHUNKS, NCHUNK, K, DBLK, 128).transpose(0, 4, 2, 3, 1)
        ).reshape(CHUNKS, 128, K * DBLK * NCHUNK)
        in_maps.append({"wt": pack, "fnT": fnT_f16})

    _install_profile_hook()
    res = None
    for attempt in range(3):
        try:
            res = run_bass_kernel_spmd(nc, in_maps, list(range(NCORES)))
            break
        except Exception:
            # Rare transient NRT_EXEC_UNIT_UNRECOVERABLE; retry fresh.
            if attempt == 2:
                raise
    LAST_RESULT = res
    outp = np.concatenate([res.results[m]["out"] for m in range(NCORES)], axis=1)

    # --- host: exact margin value at each label column ---
    idx3 = (lab[:, None] * K + np.arange(K)[None, :]).reshape(-1)
    W3 = w[idx3]
    W3 = W3 / np.linalg.norm(W3, axis=1, keepdims=True)
    c = np.einsum("bkd,bd->bk", W3.reshape(B, K, D), fn).max(axis=1)
    ms = marg[lab]
    sine = np.sqrt(np.maximum(0.0, 1.0 - c * c))
    phi = np.where(
        c > np.cos(np.pi - ms),
        c * np.cos(ms) - sine * np.sin(ms),
        c - np.sin(np.pi - ms) * ms,
    )
    outp[np.arange(B), lab] = (phi * S).astype(np.float32)
    return outp


# revision 24
# speedup vs baseline: 1.1743x; 1.1743x over previous
"""ArcFace loss with adaptive margins and subcenters, distributed over 8 TRN2 cores.

Problem: features [512, 512] f32, weight [300000, 512] f32 (100000 classes x 3
subcenters), margins [100000] f32, labels [512] int. Output [512, 100000] f32:
S * max_k cos(f, w_{c,k}) everywhere, with the ArcFace margin phi at each
sample's label column.

Strategy (classifier/model parallel, per the class-sharding hint):
  - Host: L2-normalize features and weights, fold the scale S into the weight,
    cast to fp16, and pack each core's 12500-class shard into a DMA-friendly
    layout. Also compute (exactly, in f32) the per-sample label-column value
    phi, since that needs only 512 dot products.
  - Device (x8, no collectives needed): stream the packed weight shard from
    HBM, 3 GEMMs per class-chunk (one per subcenter) accumulating over the
    D=512 contraction in PSUM, elementwise max over the 3 subcenters on the
    vector engine, write the S-scaled cosine shard back to HBM.
  - Host: concatenate the 8 [512, 12500] shards and overwrite the 512 label
    entries with S*phi.

Per-core roofline: 9.8 GMAC -> ~253 us on the PE at fp16; 64 MB of HBM
traffic -> ~180 us. PE-bound at ~260 us if DMA overlaps.
"""

import numpy as np

B = 512            # batch
D = 512            # in_features
C = 100000         # n_classes
K = 3              # subcenters
S = 30.0           # ArcFace scale
NCORES = 8
CPC = C // NCORES  # classes per core = 12500
NCHUNK = 500       # output columns per PSUM tile
CHUNKS = CPC // NCHUNK   # 25
NB = B // 128      # 4 row blocks of the batch
DBLK = D // 128    # 4 contraction blocks

_CACHE = {}
LAST_RESULT = None  # BassKernelResults of the most recent run (for profiling)


def _install_profile_hook():
    """Make `antenv.axon_hooks` importable (concourse imports it when tracing
    is requested via BASS_TRACE) and register the NTFF hook if available."""
    import sys
    import types
    try:
        import antenv
    except ImportError:
        return
    if getattr(antenv, "axon_hooks", None) is not None:
        return
    mod = types.ModuleType("antenv.axon_hooks")
    _hook = [None]
    mod.set_axon_ntff_profile_hook = lambda h: _hook.__setitem__(0, h)
    mod.get_axon_ntff_profile_hook = lambda: _hook[0]
    sys.modules["antenv.axon_hooks"] = mod
    antenv.axon_hooks = mod
    try:
        from trn_agent_boot.trn_boot import _ntff_profile_via_ctypes
        hook = _ntff_profile_via_ctypes("/opt/axon/libaxon_pjrt.so")
        if hook is not None:
            mod.set_axon_ntff_profile_hook(hook)
    except Exception:
        pass


def _build_nc():
    if "nc" in _CACHE:
        return _CACHE["nc"]
    import concourse.bacc as bacc
    import concourse.tile as tile
    from concourse import mybir

    BF = mybir.dt.float16
    F32 = mybir.dt.float32

    nc = bacc.Bacc("TRN2", target_bir_lowering=False, debug=False, num_devices=NCORES)
    # Packed weight shard: wt[q][p][(k*DBLK+d)*NCHUNK + j] = S * wn[3*(c0+q*500+j)+k, d*128+p]
    wt = nc.dram_tensor("wt", [CHUNKS, 128, K * DBLK * NCHUNK], BF, kind="ExternalInput")
    # Normalized features, transposed: fnT[d][p][b] = fn[b, d*128+p]
    fnT = nc.dram_tensor("fnT", [DBLK, 128, B], BF, kind="ExternalInput")
    out = nc.dram_tensor("out", [B, CPC], F32, kind="ExternalOutput")

    with tile.TileContext(nc, trace_sim=False) as tc:
        with tc.tile_pool(name="fp", bufs=1) as fpool, \
             tc.tile_pool(name="wp", bufs=4) as wpool, \
             tc.tile_pool(name="op", bufs=3) as opool, \
             tc.tile_pool(name="tp", bufs=4) as tpool, \
             tc.tile_pool(name="pp", bufs=2, space="PSUM") as ppool:
            f_sb = fpool.tile([128, DBLK * B], BF)
            for q in range(CHUNKS):
                w_sb = wpool.tile([128, K * DBLK * NCHUNK], BF)
                if q == 0:
                    # Split the first chunk's load into per-(k,d) slices,
                    # issued in the order the matmuls consume them (d-outer,
                    # k-inner, interleaved with the feature tiles) across
                    # both HWDGE rings, so the first matmul only waits for
                    # its own 128 KB slice.
                    wt3 = wt[q].rearrange("p (k d j) -> p k d j", k=K, d=DBLK)
                    for d in range(DBLK):
                        nc.scalar.dma_start(f_sb[:, d * B:(d + 1) * B], fnT[d])
                        for k in range(K):
                            j = k * DBLK + d
                            eng = nc.sync if k != 1 else nc.scalar
                            eng.dma_start(
                                w_sb[:, j * NCHUNK:(j + 1) * NCHUNK], wt3[:, k, d]
                            )
                elif q == 1:
                    # chunk 1 streams on the scalar ring, in parallel with
                    # chunk 0's slices on sync's
                    nc.scalar.dma_start(w_sb[:], wt[q])
                else:
                    nc.sync.dma_start(w_sb[:], wt[q])
                for b in range(NB):
                    ps = [
                        ppool.tile([128, NCHUNK], F32, tag=f"ps{k}", name=f"ps{k}")
                        for k in range(K)
                    ]
                    # d-outer / k-inner: the stationary operand (features)
                    # is reused across the 3 subcenter matmuls.
                    for d in range(DBLK):
                        lh = f_sb[:, d * B + b * 128: d * B + (b + 1) * 128]
                        for k in range(K):
                            rh = w_sb[:, (k * DBLK + d) * NCHUNK:(k * DBLK + d + 1) * NCHUNK]
                            nc.tensor.matmul(
                                ps[k][:], lh, rh,
                                start=(d == 0), stop=(d == DBLK - 1),
                                skip_group_check=True,
                            )
                    # DVE can't read two PSUM banks in one op; stage k=0
                    # through SBUF on the (otherwise idle) scalar engine.
                    t0 = tpool.tile([128, NCHUNK], F32, tag="t0", name="t0")
                    nc.scalar.copy(t0[:], ps[0][:])
                    t01 = tpool.tile([128, NCHUNK], F32, tag="t01", name="t01")
                    nc.vector.tensor_max(t01[:], t0[:], ps[1][:])
                    ob = opool.tile([128, NCHUNK], F32, tag=f"ob{b}", name=f"ob{b}")
                    nc.vector.tensor_max(ob[:], t01[:], ps[2][:])
                    # Output stores go on the scalar engine's HWDGE ring so
                    # they don't queue ahead of weight prefetches on sync's.
                    nc.scalar.dma_start(
                        out[b * 128:(b + 1) * 128, q * NCHUNK:(q + 1) * NCHUNK],
                        ob[:],
                    )
    nc.compile()
    _CACHE["nc"] = nc
    return nc


def _to_f16(x):
    # fp16 storage/compute: same TensorE rate and DMA bytes as bf16, but a
    # 10-bit mantissa -> ~4x less quantization error. All values here are
    # bounded by S=30, far inside fp16 range.
    return np.asarray(x, np.float32).astype(np.float16)


def kernel(features, weight, margins, labels):
    global LAST_RESULT
    from concourse.bass_utils import run_bass_kernel_spmd

    feats = np.asarray(features, np.float32)
    w = np.asarray(weight, np.float32)
    marg = np.asarray(margins, np.float32)
    lab = np.asarray(labels).astype(np.int64)

    nc = _build_nc()

    # --- host prep: normalize, fold S, pack per core ---
    fn = feats / np.linalg.norm(feats, axis=1, keepdims=True)
    fnT_f16 = np.ascontiguousarray(_to_f16(fn.T).reshape(DBLK, 128, B))

    R = CPC * K  # weight rows per core
    in_maps = []
    for m in range(NCORES):
        rows = w[m * R:(m + 1) * R]
        nrm = np.sqrt(np.einsum("ij,ij->i", rows, rows, dtype=np.float32))
        arr = _to_f16(rows * (S / nrm)[:, None])
        # [3c+k, d] -> [q, p, k, d, j]
        pack = np.ascontiguousarray(
            arr.reshape(CHUNKS, NCHUNK, K, DBLK, 128).transpose(0, 4, 2, 3, 1)
        ).reshape(CHUNKS, 128, K * DBLK * NCHUNK)
        in_maps.append({"wt": pack, "fnT": fnT_f16})

    _install_profile_hook()
    res = None
    for attempt in range(3):
        try:
            res = run_bass_kernel_spmd(nc, in_maps, list(range(NCORES)))
            break
        except Exception:
            # Rare transient NRT_EXEC_UNIT_UNRECOVERABLE; retry fresh.
            if attempt == 2:
                raise
    LAST_RESULT = res
    outp = np.concatenate([res.results[m]["out"] for m in range(NCORES)], axis=1)

    # --- host: exact margin value at each label column ---
    idx3 = (lab[:, None] * K + np.arange(K)[None, :]).reshape(-1)
    W3 = w[idx3]
    W3 = W3 / np.linalg.norm(W3, axis=1, keepdims=True)
    c = np.einsum("bkd,bd->bk", W3.reshape(B, K, D), fn).max(axis=1)
    ms = marg[lab]
    sine = np.sqrt(np.maximum(0.0, 1.0 - c * c))
    phi = np.where(
        c > np.cos(np.pi - ms),
        c * np.cos(ms) - sine * np.sin(ms),
        c - np.sin(np.pi - ms) * ms,
    )
    outp[np.arange(B), lab] = (phi * S).astype(np.float32)
    return outp
